# revision 13
# baseline (speedup 1.0000x reference)
"""Trainium2 Bass kernel for nn_DoubleLayer (e3nn-style double tensor-product layer).

Math per row b (layout x = [s(8) | v(8 vec channels, u-major xyz)]):
  layer(s, v; W) with irreps (ms x 0e + mv x 1o) -> (mw x 0e + mw x 1o):
    out_s[w]   = c0*(sum_uv s_u s_v Wss[u,v,w] + 1/sqrt3 * sum_uvi v_ui v_vi Wvv0[u,v,w])
    out_v[w,k] = c1*(1/sqrt3*(sum_uv s_u v_vk Wsv[u,v,w] + v_uk s_v Wvs[u,v,w])
                 + 1/sqrt6 * sum eps_ijk v_ui v_vj Wvv1[u,v,w])
  x -> tanh(s),v -> L1 -> si_norm -> tv_norm -> L2 -> si_norm -> sigmoid(s).

Kernel strategy (pure data parallel over 8 cores, 32768 rows/core):
  For each 128-row chunk (batch rows on SBUF partitions):
    1. PE transpose z [128, nf] -> z^T [nf, 128] (stationary for pass-1)
    2. PE pass-1: z^T @ bundle[nf, 1024] -> per-row intermediates M in PSUM.
       Bundle columns are host-packed weighted contractions:
         A[u,w]   = sum_f s_f * c0*Wss[u,f,w]
         B[u,i,w] = sum_f v_fi * c0/sqrt3*Wvv0[u,f,w]
         C[m,w]   = sum_f s_f * c1/sqrt3*(Wsv[f,m,w]+Wvs[m,f,w])
         D[u,j,w] = sum_f v_fj * c1/sqrt6*Wvv1[u,f,w]
    3. DVE/POOL products: s_u*A, v_ui*B, v_mk*C, +/- v_ui*D (broadcast APs of z)
       written into per-output contiguous segments.
    4. DVE segmented tensor_reduce -> out_s [128, mw], out_v [128, 3mw].
  Norms (si_norm/tv_norm, per-row over channels) are batched across T=8 chunks.
"""

import sys
import numpy as np

for _p in ("/opt/trn_rl_repo",):
    if _p not in sys.path:
        sys.path.append(_p)

MI, MH, MO = 8, 16, 8
NB = 262144
NCORES = 8
ROWS_PER_CORE = NB // NCORES
P = 128
T = 8                      # chunks per macro tile
MACRO = P * T              # 1024 rows
EPS_SI = 1e-9
EPS_TV = 1e-6
TINY = 1e-12


def _build_bundle(ms, mv, mw, Wss, Wvv0, Wsv, Wvs, Wvv1):
    """Pack weighted-contraction bundle [nf, 1024], nf = ms + 3*mv.

    z feature layout: f in [0, ms) = s_f ; f = ms + 3*u + i = v[u, i].
    Column regions (all w-innermost):
      A: col = u*mw + w                      rows: s       val c0*Wss[u, f, w]
      B: col = oB + u*3*mw + i*mw + w        rows: v[:, i] val c0/sqrt3*Wvv0[u, f, w]
      C: col = oC + m*mw + w                 rows: s       val c1/sqrt3*(Wsv[f,m,w]+Wvs[m,f,w])
      D: col = oD + u*3*mw + j*mw + w        rows: v[:, j] val c1/sqrt6*Wvv1[u, f, w]
    """
    nf = ms + 3 * mv
    c0 = (ms * ms + mv * mv) ** -0.5
    c1 = (3.0 / (2 * ms * mv + mv * mv)) ** 0.5
    inv3 = 3.0 ** -0.5
    inv6 = 6.0 ** -0.5
    oB = ms * mw
    oC = oB + 3 * mv * mw
    oD = oC + mv * mw
    ncols = oD + 3 * mv * mw
    B = np.zeros((nf, ncols), np.float32)
    for u in range(ms):
        for w in range(mw):
            B[0:ms, u * mw + w] = c0 * Wss[u, :, w]
    vrows = ms + 3 * np.arange(mv)
    for u in range(mv):
        for i in range(3):
            for w in range(mw):
                B[vrows + i, oB + u * 3 * mw + i * mw + w] = c0 * inv3 * Wvv0[u, :, w]
    for m in range(mv):
        for w in range(mw):
            B[0:ms, oC + m * mw + w] = c1 * inv3 * (Wsv[:, m, w] + Wvs[m, :, w])
    for u in range(mv):
        for j in range(3):
            for w in range(mw):
                B[vrows + j, oD + u * 3 * mw + j * mw + w] = c1 * inv6 * Wvv1[u, :, w]
    return B


# ---------------------------------------------------------------------------
# numpy emulation of the device pipeline (for bundle/layout validation)
# ---------------------------------------------------------------------------

def _np_layer(z, bundle, ms, mv, mw):
    nf = ms + 3 * mv
    oB = ms * mw
    oC = oB + 3 * mv * mw
    oD = oC + mv * mw
    M = z @ bundle  # [n, 1024]
    n = z.shape[0]
    s = z[:, 0:ms]
    v = z[:, ms:nf].reshape(n, mv, 3)
    A = M[:, 0:oB].reshape(n, ms, mw)
    Bm = M[:, oB:oC].reshape(n, mv, 3, mw)
    C = M[:, oC:oD].reshape(n, mv, mw)
    D = M[:, oD:].reshape(n, mv, 3, mw)
    out_s = np.einsum('nu,nuw->nw', s, A) + np.einsum('nui,nuiw->nw', v, Bm)
    out_v = np.einsum('nmk,nmw->nwk', v, C)
    for k in range(3):
        j1, i1 = (k + 2) % 3, (k + 1) % 3
        j2, i2 = (k + 1) % 3, (k + 2) % 3
        out_v[:, :, k] += np.einsum('nu,nuw->nw', v[:, :, i1], D[:, :, j1, :])
        out_v[:, :, k] -= np.einsum('nu,nuw->nw', v[:, :, i2], D[:, :, j2, :])
    return out_s, out_v.reshape(n, mw * 3)


def _np_si_norm(ys, yv3):
    n, mh = ys.shape
    std_s = ys.std(axis=1, ddof=1)
    ys = ys / (std_s[:, None] + EPS_SI)
    yv = yv3.reshape(n, -1, 3)
    norm1 = np.sqrt((yv ** 2).sum(axis=2) + EPS_SI)
    std_v = norm1.std(axis=1, ddof=1)
    yv = yv / (std_v[:, None, None] + EPS_SI)
    return ys, yv.reshape(n, -1)


def _np_tv_norm(xs, xv3):
    n = xs.shape[0]
    xs = xs / np.sqrt((xs ** 2).sum(axis=1, keepdims=True) + EPS_TV)
    xv = xv3.reshape(n, -1, 3)
    norm1 = np.sqrt((xv ** 2).sum(axis=1) + EPS_TV)   # [n, 3]
    nm = norm1.mean(axis=1)
    xv = xv / (nm[:, None, None] + EPS_TV)
    return xs, xv.reshape(n, -1)


def reference_numpy(x, B1, B2):
    xs = np.tanh(x[:, :MI])
    z1 = np.concatenate([xs, x[:, MI:]], axis=1)
    ys, yv = _np_layer(z1, B1, MI, MI, MH)
    ys, yv = _np_si_norm(ys, yv)
    ys, yv = _np_tv_norm(ys, yv)
    z2 = np.concatenate([ys, yv], axis=1)
    zs, zv = _np_layer(z2, B2, MH, MH, MO)
    zs, zv = _np_si_norm(zs, zv)
    zs = 1.0 / (1.0 + np.exp(-zs))
    return np.concatenate([zs, zv], axis=1).astype(np.float32)


# ---------------------------------------------------------------------------
# device kernel
# ---------------------------------------------------------------------------

_PROGRAM_CACHE = {}


def _emit_layer_set(nc, pools, src_ap_full, t0, S, nf_stride, vneg_full, bundle_rep,
                    ztall, ms, mv, mw, ys_macro, yv_macro):
    """One set of S (<=3) 128-row chunks of one tensor-product layer.

    src_ap_full: [128, T*nf_stride] SBUF macro tile holding z per chunk
    vneg_full:   [128, T*3mv] negated v features macro tile
    ys_macro:    [128, T*mw] tile; yv_macro: [128, T*3mw]
    """
    import concourse.mybir as mybir
    f32 = mybir.dt.float32
    nf = ms + 3 * mv
    q_s = ms + 3 * mv
    oC = mw * q_s
    oD = oC + mv * mw
    scols = mw * q_s

    pm = pools["psum_m"].tile([P, 3 * 1024], f32, tag="pm")
    npack = P // nf
    for c in range(S):
        t = t0 + c
        r0 = nf * (t % npack)
        lhsT = ztall[r0:r0 + nf, (t // npack) * P:(t // npack) * P + P]
        rhs0 = bundle_rep[r0:r0 + nf, 0:512]
        rhs1 = bundle_rep[r0:r0 + nf, 512:1024]
        tp = (r0, 0)
        nc.tensor.matmul(pm[:, c * 1024:c * 1024 + 512], lhsT, rhs0,
                         start=True, stop=True, tile_position=tp)
        nc.tensor.matmul(pm[:, c * 1024 + 512:c * 1024 + 1024], lhsT, rhs1,
                         start=True, stop=True, tile_position=tp)

    prod = pools["prod"].tile([P, 3 * 1664], f32, tag="prod")
    pm3 = pm[:, 0:S * 1024].rearrange("p (c n) -> p c n", c=S)
    pr3 = prod[:, 0:S * 1664].rearrange("p (c r) -> p c r", c=S)
    zsp = src_ap_full[:, t0 * nf_stride:(t0 + S - 1) * nf_stride + nf] \
        if nf_stride != nf else src_ap_full[:, t0 * nf:(t0 + S) * nf]
    # build [p, c, f] view of z across the set (chunk stride nf_stride)
    zc = src_ap_full[:, t0 * nf_stride:(t0 + S) * nf_stride].rearrange(
        "p (c f) -> p c f", c=S)[:, :, 0:nf]
    vc = zc[:, :, ms:nf]
    vnegc = vneg_full[:, t0 * 3 * mv:(t0 + S) * 3 * mv].rearrange(
        "p (c f) -> p c f", c=S)

    spart = pr3[:, :, 0:scols].rearrange("p c (w q) -> p c w q", w=mw)
    vpart = pr3[:, :, scols:1664].rearrange("p c (w k q) -> p c w k q", w=mw, k=3)

    # merged A+B products: z_q * M[w, q]
    nc.vector.tensor_mul(
        spart,
        pm3[:, :, 0:scols].rearrange("p c (w q) -> p c w q", w=mw),
        zc.unsqueeze(2).broadcast_to([P, S, mw, q_s]),
    )
    # C products: v[m,k] * C[m,w]  (one op per k: ISA caps free dims at 3)
    Cv = pm3[:, :, oC:oD].rearrange("p c (m w) -> p c w m", m=mv)
    vck = vc.rearrange("p c (m k) -> p c k m", m=mv)
    for k in range(3):
        nc.vector.tensor_mul(
            vpart[:, :, :, k, 0:mv],
            Cv,
            vck[:, :, k].unsqueeze(2).broadcast_to([P, S, mw, mv]),
        )
    # D products
    Dv = pm3[:, :, oD:1024].rearrange("p c (u j w) -> p c j w u", u=mv, j=3)
    vpos = vc.rearrange("p c (u i) -> p c i u", u=mv)
    vneg = vnegc.rearrange("p c (u i) -> p c i u", u=mv)
    for k in range(3):
        j1, i1 = (k + 2) % 3, (k + 1) % 3
        j2, i2 = (k + 1) % 3, (k + 2) % 3
        nc.vector.tensor_mul(
            vpart[:, :, :, k, mv:2 * mv],
            Dv[:, :, j1],
            vpos[:, :, i1].unsqueeze(2).broadcast_to([P, S, mw, mv]),
        )
        nc.vector.tensor_mul(
            vpart[:, :, :, k, 2 * mv:3 * mv],
            Dv[:, :, j2],
            vneg[:, :, i2].unsqueeze(2).broadcast_to([P, S, mw, mv]),
        )

    nc.vector.tensor_reduce(
        ys_macro[:, t0 * mw:(t0 + S) * mw].rearrange("p (c w) -> p c w", c=S),
        spart, axis=mybir.AxisListType.X, op=mybir.AluOpType.add)
    nc.vector.tensor_reduce(
        yv_macro[:, t0 * 3 * mw:(t0 + S) * 3 * mw].rearrange("p (c wk) -> p c wk", c=S),
        vpart.rearrange("p c w k q -> p c (w k) q"),
        axis=mybir.AxisListType.X, op=mybir.AluOpType.add)


def _emit_program(nc, tc, x_d, b1_d, b2_d, out_d, rows, repeat=1):
    import concourse.mybir as mybir
    from concourse.masks import make_identity
    AF = mybir.ActivationFunctionType
    ALU = mybir.AluOpType
    AX = mybir.AxisListType
    f32 = mybir.dt.float32
    from contextlib import ExitStack

    nmacro = rows // MACRO
    ctx = ExitStack()
    with ctx:
        cpool = ctx.enter_context(tc.tile_pool(name="const", bufs=1))
        ident = cpool.tile([P, P], f32)
        make_identity(nc, ident[:])

        def _bias_tile(val, idx=[0]):
            bt = cpool.tile([P, 1], f32, tag=f"bias{idx[0]}")
            idx[0] += 1
            nc.gpsimd.memset(bt[:], float(val))
            return bt[:]

        b_tiny = _bias_tile(TINY)
        b_si = _bias_tile(EPS_SI)
        b_tv = _bias_tile(EPS_TV)
        b_v1 = _bias_tile(MH * EPS_SI / (MH - 1) + TINY)
        b_v2 = _bias_tile(MO * EPS_SI / (MO - 1) + TINY)
        b1_sb = cpool.tile([P, 1024], f32)
        for r in range(4):
            nc.sync.dma_start(out=b1_sb[32 * r:32 * (r + 1), :], in_=b1_d[:])
        b2_sb = cpool.tile([P, 1024], f32)
        for r in range(2):
            nc.sync.dma_start(out=b2_sb[64 * r:64 * (r + 1), :], in_=b2_d[:])

        io_pool = ctx.enter_context(tc.tile_pool(name="io", bufs=2))
        mid_pool = ctx.enter_context(tc.tile_pool(name="mid", bufs=2))
        nrm_pool = ctx.enter_context(tc.tile_pool(name="nrm", bufs=2))
        zt_pool = ctx.enter_context(tc.tile_pool(name="zt", bufs=2))
        prod_pool = ctx.enter_context(tc.tile_pool(name="prod", bufs=2))
        psum_t = ctx.enter_context(tc.tile_pool(name="psum_t", bufs=2, space="PSUM"))
        psum_m = ctx.enter_context(tc.tile_pool(name="psum_m", bufs=1, space="PSUM"))
        pools = {"psum_t": psum_t, "psum_m": psum_m, "zt": zt_pool, "prod": prod_pool}

        for m in list(range(nmacro)) * repeat:
            xm = io_pool.tile([P, T * 32], f32, tag="xm")
            xview = x_d[m * MACRO:(m + 1) * MACRO, :].rearrange("(t p) f -> p t f", p=P)
            nc.sync.dma_start(out=xm[:].rearrange("p (t f) -> p t f", f=32), in_=xview)
            # tanh on scalar features (in place)
            xs_view = xm[:].rearrange("p (t f) -> p t f", f=32)[:, :, 0:MI]
            nc.scalar.activation(xs_view, xs_view, AF.Tanh)
            # negated vector features for the minus-branch of the cross products
            zneg1 = io_pool.tile([P, T * 24], f32, tag="zneg1")
            nc.vector.tensor_scalar_mul(
                zneg1[:].rearrange("p (t q) -> p t q", q=24),
                xm[:].rearrange("p (t f) -> p t f", f=32)[:, :, MI:32],
                -1.0,
            )

            ztall1 = zt_pool.tile([P, (T // 4) * P], f32, tag="ztall1")
            for g in range(T // 4):
                pt = psum_t.tile([P, P], f32, tag="pt")
                nc.tensor.transpose(pt[:], xm[:, g * P:(g + 1) * P], ident[:])
                nc.vector.tensor_copy(ztall1[:, g * P:(g + 1) * P], pt[:])
            ys8 = nrm_pool.tile([P, T * MH], f32, tag="ys8")
            yv8 = nrm_pool.tile([P, T * 3 * MH], f32, tag="yv8")
            t0 = 0
            while t0 < T:
                S = min(3, T - t0)
                _emit_layer_set(nc, pools, xm[:], t0, S, 32, zneg1[:],
                                b1_sb[:], ztall1[:], MI, MI, MH, ys8[:], yv8[:])
                t0 += S

            # ---- si_norm(1) + tv_norm, batched over T chunks ----
            mh = MH
            ys8v = ys8[:].rearrange("p (t f) -> p t f", f=mh)
            yv8v = yv8[:].rearrange("p (t f) -> p t f", f=3 * mh)
            sq_s = nrm_pool.tile([P, T * mh], f32, tag="sq_s")
            nc.scalar.activation(sq_s[:], ys8[:], AF.Square)
            sumsq_s = nrm_pool.tile([P, T], f32, tag="n1a")
            nc.vector.tensor_reduce(sumsq_s[:], sq_s[:].rearrange("p (t f) -> p t f", f=mh),
                                    axis=AX.X, op=ALU.add)
            sum_s = nrm_pool.tile([P, T], f32, tag="n1b")
            nc.vector.tensor_reduce(sum_s[:], ys8v, axis=AX.X, op=ALU.add)
            s2 = nrm_pool.tile([P, T], f32, tag="n1c")
            nc.scalar.activation(s2[:], sum_s[:], AF.Square, scale=float(mh) ** -0.5)
            varnum = nrm_pool.tile([P, T], f32, tag="n1d")
            nc.vector.tensor_sub(varnum[:], sumsq_s[:], s2[:])
            std_s = nrm_pool.tile([P, T], f32, tag="n1e")
            nc.scalar.activation(std_s[:], varnum[:], AF.Sqrt,
                                 scale=1.0 / (mh - 1), bias=b_tiny)
            stde_s = nrm_pool.tile([P, T], f32, tag="n1f")
            nc.vector.tensor_scalar_add(stde_s[:], std_s[:], EPS_SI)
            inv_s = nrm_pool.tile([P, T], f32, tag="n1g")
            nc.vector.reciprocal(inv_s[:], stde_s[:])

            sq_v = nrm_pool.tile([P, T * 3 * mh], f32, tag="sq_v")
            nc.scalar.activation(sq_v[:], yv8[:], AF.Square)
            n2u = nrm_pool.tile([P, T * mh], f32, tag="n2u")
            nc.vector.tensor_reduce(
                n2u[:], sq_v[:].rearrange("p (tu k) -> p tu k", k=3),
                axis=AX.X, op=ALU.add)
            norm1 = nrm_pool.tile([P, T * mh], f32, tag="norm1")
            nc.scalar.activation(norm1[:], n2u[:], AF.Sqrt, bias=b_si)
            rn = nrm_pool.tile([P, T], f32, tag="n1h")
            nc.vector.tensor_reduce(rn[:], n2u[:].rearrange("p (t u) -> p t u", u=mh),
                                    axis=AX.X, op=ALU.add)
            sum_n = nrm_pool.tile([P, T], f32, tag="n1i")
            nc.vector.tensor_reduce(sum_n[:], norm1[:].rearrange("p (t u) -> p t u", u=mh),
                                    axis=AX.X, op=ALU.add)
            s2n = nrm_pool.tile([P, T], f32, tag="n1j")
            nc.scalar.activation(s2n[:], sum_n[:], AF.Square, scale=float(mh) ** -0.5)
            varn = nrm_pool.tile([P, T], f32, tag="n1k")
            nc.vector.tensor_sub(varn[:], rn[:], s2n[:])
            std_v = nrm_pool.tile([P, T], f32, tag="n1l")
            nc.scalar.activation(std_v[:], varn[:], AF.Sqrt, scale=1.0 / (mh - 1),
                                 bias=b_v1)
            stde_v = nrm_pool.tile([P, T], f32, tag="n1m")
            nc.vector.tensor_scalar_add(stde_v[:], std_v[:], EPS_SI)
            inv_v = nrm_pool.tile([P, T], f32, tag="n1n")
            nc.vector.reciprocal(inv_v[:], stde_v[:])

            # tv_norm scalars
            invs2 = nrm_pool.tile([P, T], f32, tag="n1o")
            nc.scalar.activation(invs2[:], inv_s[:], AF.Square)
            q_s = nrm_pool.tile([P, T], f32, tag="n1p")
            nc.vector.tensor_mul(q_s[:], sumsq_s[:], invs2[:])
            rt_s = nrm_pool.tile([P, T], f32, tag="n1q")
            nc.scalar.activation(rt_s[:], q_s[:], AF.Sqrt, bias=b_tv)
            invt_s = nrm_pool.tile([P, T], f32, tag="n1r")
            nc.vector.reciprocal(invt_s[:], rt_s[:])
            scale_s = nrm_pool.tile([P, T], f32, tag="n1s")
            nc.vector.tensor_mul(scale_s[:], inv_s[:], invt_s[:])

            ni_raw = nrm_pool.tile([P, T * 3], f32, tag="n1t")
            nc.vector.tensor_reduce(
                ni_raw[:].rearrange("p (t i) -> p t i", i=3),
                sq_v[:].rearrange("p (t u i) -> p t i u", u=mh, i=3),
                axis=AX.X, op=ALU.add)
            iv2 = nrm_pool.tile([P, T], f32, tag="n1u")
            nc.scalar.activation(iv2[:], inv_v[:], AF.Square)
            ni_tv = nrm_pool.tile([P, T * 3], f32, tag="n1v")
            nc.vector.tensor_mul(
                ni_tv[:].rearrange("p (t i) -> p t i", i=3),
                ni_raw[:].rearrange("p (t i) -> p t i", i=3),
                iv2[:].unsqueeze(2).broadcast_to([P, T, 3]))
            n1tv = nrm_pool.tile([P, T * 3], f32, tag="n1w")
            nc.scalar.activation(n1tv[:], ni_tv[:], AF.Sqrt, bias=b_tv)
            nm3 = nrm_pool.tile([P, T], f32, tag="n1x")
            nc.vector.tensor_reduce(nm3[:], n1tv[:].rearrange("p (t i) -> p t i", i=3),
                                    axis=AX.X, op=ALU.add)
            nme = nrm_pool.tile([P, T], f32, tag="n1y")
            nc.vector.tensor_scalar(nme[:], nm3[:], 1.0 / 3.0, EPS_TV,
                                    op0=ALU.mult, op1=ALU.add)
            invtv = nrm_pool.tile([P, T], f32, tag="n1z")
            nc.vector.reciprocal(invtv[:], nme[:])
            scale_v = nrm_pool.tile([P, T], f32, tag="n1A")
            nc.vector.tensor_mul(scale_v[:], inv_v[:], invtv[:])

            # apply scales -> zmid [128, T*64]; also negated v for L2 D-products
            zmid = mid_pool.tile([P, T * 64], f32, tag="zmid")
            zmv = zmid[:].rearrange("p (t f) -> p t f", f=64)
            nc.vector.tensor_mul(
                zmv[:, :, 0:MH], ys8v,
                scale_s[:].unsqueeze(2).broadcast_to([P, T, MH]))
            nc.vector.tensor_mul(
                zmv[:, :, MH:64], yv8v,
                scale_v[:].unsqueeze(2).broadcast_to([P, T, 3 * MH]))
            zneg2 = mid_pool.tile([P, T * 48], f32, tag="zneg2")
            nscale_v = nrm_pool.tile([P, T], f32, tag="n1B")
            nc.vector.tensor_scalar_mul(nscale_v[:], scale_v[:], -1.0)
            nc.vector.tensor_mul(
                zneg2[:].rearrange("p (t f) -> p t f", f=48), yv8v,
                nscale_v[:].unsqueeze(2).broadcast_to([P, T, 48]))

            # ---- layer 2 ----
            ztall2 = zt_pool.tile([P, (T // 2) * P], f32, tag="ztall2")
            for g in range(T // 2):
                pt = psum_t.tile([P, P], f32, tag="pt")
                nc.tensor.transpose(pt[:], zmid[:, g * P:(g + 1) * P], ident[:])
                nc.vector.tensor_copy(ztall2[:, g * P:(g + 1) * P], pt[:])
            zs8 = nrm_pool.tile([P, T * MO], f32, tag="zs8")
            zv8 = nrm_pool.tile([P, T * 3 * MO], f32, tag="zv8")
            t0 = 0
            while t0 < T:
                S = min(3, T - t0)
                _emit_layer_set(nc, pools, zmid[:], t0, S, 64, zneg2[:],
                                b2_sb[:], ztall2[:], MH, MH, MO, zs8[:], zv8[:])
                t0 += S

            # ---- si_norm(2) ----
            mo = MO
            zs8v = zs8[:].rearrange("p (t f) -> p t f", f=mo)
            zv8v = zv8[:].rearrange("p (t f) -> p t f", f=3 * mo)
            sq_s2 = nrm_pool.tile([P, T * mo], f32, tag="sq_s2")
            nc.scalar.activation(sq_s2[:], zs8[:], AF.Square)
            sumsq2 = nrm_pool.tile([P, T], f32, tag="n2a")
            nc.vector.tensor_reduce(sumsq2[:], sq_s2[:].rearrange("p (t f) -> p t f", f=mo),
                                    axis=AX.X, op=ALU.add)
            sum2 = nrm_pool.tile([P, T], f32, tag="n2b")
            nc.vector.tensor_reduce(sum2[:], zs8v, axis=AX.X, op=ALU.add)
            s22 = nrm_pool.tile([P, T], f32, tag="n2c")
            nc.scalar.activation(s22[:], sum2[:], AF.Square, scale=float(mo) ** -0.5)
            varnum2 = nrm_pool.tile([P, T], f32, tag="n2d")
            nc.vector.tensor_sub(varnum2[:], sumsq2[:], s22[:])
            std_s2 = nrm_pool.tile([P, T], f32, tag="n2e")
            nc.scalar.activation(std_s2[:], varnum2[:], AF.Sqrt,
                                 scale=1.0 / (mo - 1), bias=b_tiny)
            stde_s2 = nrm_pool.tile([P, T], f32, tag="n2f")
            nc.vector.tensor_scalar_add(stde_s2[:], std_s2[:], EPS_SI)
            inv_s2 = nrm_pool.tile([P, T], f32, tag="n2g")
            nc.vector.reciprocal(inv_s2[:], stde_s2[:])

            sq_v2 = nrm_pool.tile([P, T * 3 * mo], f32, tag="sq_v2")
            nc.scalar.activation(sq_v2[:], zv8[:], AF.Square)
            n2u2 = nrm_pool.tile([P, T * mo], f32, tag="n2u2")
            nc.vector.tensor_reduce(n2u2[:], sq_v2[:].rearrange("p (tu k) -> p tu k", k=3),
                                    axis=AX.X, op=ALU.add)
            norm12 = nrm_pool.tile([P, T * mo], f32, tag="norm12")
            nc.scalar.activation(norm12[:], n2u2[:], AF.Sqrt, bias=b_si)
            rn2 = nrm_pool.tile([P, T], f32, tag="n2h")
            nc.vector.tensor_reduce(rn2[:], n2u2[:].rearrange("p (t u) -> p t u", u=mo),
                                    axis=AX.X, op=ALU.add)
            sum_n2 = nrm_pool.tile([P, T], f32, tag="n2i")
            nc.vector.tensor_reduce(sum_n2[:], norm12[:].rearrange("p (t u) -> p t u", u=mo),
                                    axis=AX.X, op=ALU.add)
            s2n2 = nrm_pool.tile([P, T], f32, tag="n2j")
            nc.scalar.activation(s2n2[:], sum_n2[:], AF.Square, scale=float(mo) ** -0.5)
            varn2 = nrm_pool.tile([P, T], f32, tag="n2k")
            nc.vector.tensor_sub(varn2[:], rn2[:], s2n2[:])
            std_v2 = nrm_pool.tile([P, T], f32, tag="n2l")
            nc.scalar.activation(std_v2[:], varn2[:], AF.Sqrt, scale=1.0 / (mo - 1),
                                 bias=b_v2)
            stde_v2 = nrm_pool.tile([P, T], f32, tag="n2m")
            nc.vector.tensor_scalar_add(stde_v2[:], std_v2[:], EPS_SI)
            inv_v2 = nrm_pool.tile([P, T], f32, tag="n2n")
            nc.vector.reciprocal(inv_v2[:], stde_v2[:])

            # scale, sigmoid, assemble output macro [128, T*32]
            outm = io_pool.tile([P, T * 32], f32, tag="outm")
            outv = outm[:].rearrange("p (t f) -> p t f", f=32)
            tmp_s = nrm_pool.tile([P, T * mo], f32, tag="tmp_s")
            nc.vector.tensor_mul(
                tmp_s[:].rearrange("p (t f) -> p t f", f=mo), zs8v,
                inv_s2[:].unsqueeze(2).broadcast_to([P, T, mo]))
            nc.scalar.activation(outv[:, :, 0:MO],
                                 tmp_s[:].rearrange("p (t f) -> p t f", f=mo),
                                 AF.Sigmoid)
            nc.vector.tensor_mul(
                outv[:, :, MO:32], zv8v,
                inv_v2[:].unsqueeze(2).broadcast_to([P, T, 3 * mo]))

            oview = out_d[m * MACRO:(m + 1) * MACRO, :].rearrange("(t p) f -> p t f", p=P)
            nc.sync.dma_start(out=oview, in_=outm[:].rearrange("p (t f) -> p t f", f=32))


def _build_program(rows, repeat=1):
    import concourse.bacc as bacc
    import concourse.tile as tile
    import concourse.mybir as mybir
    f32 = mybir.dt.float32

    nc = bacc.Bacc("TRN2", target_bir_lowering=False, debug=False,
                   enable_asserts=False, num_devices=NCORES)
    x_d = nc.dram_tensor("x", [rows, 32], f32, kind="ExternalInput").ap()
    b1_d = nc.dram_tensor("b1", [32, 1024], f32, kind="ExternalInput").ap()
    b2_d = nc.dram_tensor("b2", [64, 1024], f32, kind="ExternalInput").ap()
    out_d = nc.dram_tensor("out", [rows, 32], f32, kind="ExternalOutput").ap()

    with tile.TileContext(nc) as tc:
        _emit_program(nc, tc, x_d, b1_d, b2_d, out_d, rows, repeat)
    nc.compile()
    return nc


def _get_program(rows, repeat=1):
    key = (rows, repeat)
    if key not in _PROGRAM_CACHE:
        _PROGRAM_CACHE[key] = _build_program(rows, repeat)
    return _PROGRAM_CACHE[key]


def kernel(x, w1_ss, w1_vv0, w1_sv, w1_vs, w1_vv1,
           w2_ss, w2_vv0, w2_sv, w2_vs, w2_vv1, _trace=False, _repeat=1):
    from concourse import bass_utils

    x = np.asarray(x, dtype=np.float32)
    B1 = _build_bundle(MI, MI, MH, np.asarray(w1_ss), np.asarray(w1_vv0),
                       np.asarray(w1_sv), np.asarray(w1_vs), np.asarray(w1_vv1))
    B2 = _build_bundle(MH, MH, MO, np.asarray(w2_ss), np.asarray(w2_vv0),
                       np.asarray(w2_sv), np.asarray(w2_vs), np.asarray(w2_vv1))

    rows = x.shape[0] // NCORES
    nc = _get_program(rows, _repeat)
    shards = x.reshape(NCORES, rows, 32)
    in_maps = [{"x": np.ascontiguousarray(shards[i]), "b1": B1, "b2": B2}
               for i in range(NCORES)]
    res = bass_utils.run_bass_kernel_spmd(nc, in_maps, core_ids=list(range(NCORES)),
                                          trace=_trace)
    out = np.concatenate([res.results[i]["out"] for i in range(NCORES)], axis=0)
    if _trace:
        return out, res
    return out


# revision 16
# speedup vs baseline: 136.5135x; 136.5135x over previous
"""Trainium2 Bass kernel for nn_DoubleLayer (e3nn-style double tensor-product layer).

Math per row b (layout x = [s(8) | v(8 vec channels, u-major xyz)]):
  layer(s, v; W) with irreps (ms x 0e + mv x 1o) -> (mw x 0e + mw x 1o):
    out_s[w]   = c0*(sum_uv s_u s_v Wss[u,v,w] + 1/sqrt3 * sum_uvi v_ui v_vi Wvv0[u,v,w])
    out_v[w,k] = c1*(1/sqrt3*(sum_uv s_u v_vk Wsv[u,v,w] + v_uk s_v Wvs[u,v,w])
                 + 1/sqrt6 * sum eps_ijk v_ui v_vj Wvv1[u,v,w])
  x -> tanh(s),v -> L1 -> si_norm -> tv_norm -> L2 -> si_norm -> sigmoid(s).

Kernel strategy (pure data parallel over 8 cores, 32768 rows/core):
  For each 128-row chunk (batch rows on SBUF partitions):
    1. PE transpose z [128, nf] -> z^T [nf, 128] (stationary for pass-1)
    2. PE pass-1: z^T @ bundle[nf, 1024] -> per-row intermediates M in PSUM.
       Bundle columns are host-packed weighted contractions:
         A[u,w]   = sum_f s_f * c0*Wss[u,f,w]
         B[u,i,w] = sum_f v_fi * c0/sqrt3*Wvv0[u,f,w]
         C[m,w]   = sum_f s_f * c1/sqrt3*(Wsv[f,m,w]+Wvs[m,f,w])
         D[u,j,w] = sum_f v_fj * c1/sqrt6*Wvv1[u,f,w]
    3. DVE/POOL products: s_u*A, v_ui*B, v_mk*C, +/- v_ui*D (broadcast APs of z)
       written into per-output contiguous segments.
    4. DVE segmented tensor_reduce -> out_s [128, mw], out_v [128, 3mw].
  Norms (si_norm/tv_norm, per-row over channels) are batched across T=8 chunks.
"""

import sys
import numpy as np

for _p in ("/opt/trn_rl_repo",):
    if _p not in sys.path:
        sys.path.append(_p)

MI, MH, MO = 8, 16, 8
NB = 262144
NCORES = 8
ROWS_PER_CORE = NB // NCORES
P = 128
T = 8                      # chunks per macro tile
MACRO = P * T              # 1024 rows
EPS_SI = 1e-9
EPS_TV = 1e-6
TINY = 1e-12


def _build_bundle(ms, mv, mw, Wss, Wvv0, Wsv, Wvs, Wvv1):
    """Pack weighted-contraction bundle [nf, 1024], nf = ms + 3*mv.

    z feature layout: f in [0, ms) = s_f ; f = ms + 3*u + i = v[u, i].
    Column regions (all w-innermost):
      A: col = u*mw + w                      rows: s       val c0*Wss[u, f, w]
      B: col = oB + u*3*mw + i*mw + w        rows: v[:, i] val c0/sqrt3*Wvv0[u, f, w]
      C: col = oC + m*mw + w                 rows: s       val c1/sqrt3*(Wsv[f,m,w]+Wvs[m,f,w])
      D: col = oD + u*3*mw + j*mw + w        rows: v[:, j] val c1/sqrt6*Wvv1[u, f, w]
    """
    nf = ms + 3 * mv
    c0 = (ms * ms + mv * mv) ** -0.5
    c1 = (3.0 / (2 * ms * mv + mv * mv)) ** 0.5
    inv3 = 3.0 ** -0.5
    inv6 = 6.0 ** -0.5
    oB = ms * mw
    oC = oB + 3 * mv * mw
    oD = oC + mv * mw
    ncols = oD + 3 * mv * mw
    B = np.zeros((nf, ncols), np.float32)
    for u in range(ms):
        for w in range(mw):
            B[0:ms, u * mw + w] = c0 * Wss[u, :, w]
    vrows = ms + 3 * np.arange(mv)
    for u in range(mv):
        for i in range(3):
            for w in range(mw):
                B[vrows + i, oB + u * 3 * mw + i * mw + w] = c0 * inv3 * Wvv0[u, :, w]
    for m in range(mv):
        for w in range(mw):
            B[0:ms, oC + m * mw + w] = c1 * inv3 * (Wsv[:, m, w] + Wvs[m, :, w])
    for u in range(mv):
        for j in range(3):
            for w in range(mw):
                B[vrows + j, oD + u * 3 * mw + j * mw + w] = c1 * inv6 * Wvv1[u, :, w]
    return B


# ---------------------------------------------------------------------------
# numpy emulation of the device pipeline (for bundle/layout validation)
# ---------------------------------------------------------------------------

def _np_layer(z, bundle, ms, mv, mw):
    nf = ms + 3 * mv
    oB = ms * mw
    oC = oB + 3 * mv * mw
    oD = oC + mv * mw
    M = z @ bundle  # [n, 1024]
    n = z.shape[0]
    s = z[:, 0:ms]
    v = z[:, ms:nf].reshape(n, mv, 3)
    A = M[:, 0:oB].reshape(n, ms, mw)
    Bm = M[:, oB:oC].reshape(n, mv, 3, mw)
    C = M[:, oC:oD].reshape(n, mv, mw)
    D = M[:, oD:].reshape(n, mv, 3, mw)
    out_s = np.einsum('nu,nuw->nw', s, A) + np.einsum('nui,nuiw->nw', v, Bm)
    out_v = np.einsum('nmk,nmw->nwk', v, C)
    for k in range(3):
        j1, i1 = (k + 2) % 3, (k + 1) % 3
        j2, i2 = (k + 1) % 3, (k + 2) % 3
        out_v[:, :, k] += np.einsum('nu,nuw->nw', v[:, :, i1], D[:, :, j1, :])
        out_v[:, :, k] -= np.einsum('nu,nuw->nw', v[:, :, i2], D[:, :, j2, :])
    return out_s, out_v.reshape(n, mw * 3)


def _np_si_norm(ys, yv3):
    n, mh = ys.shape
    std_s = ys.std(axis=1, ddof=1)
    ys = ys / (std_s[:, None] + EPS_SI)
    yv = yv3.reshape(n, -1, 3)
    norm1 = np.sqrt((yv ** 2).sum(axis=2) + EPS_SI)
    std_v = norm1.std(axis=1, ddof=1)
    yv = yv / (std_v[:, None, None] + EPS_SI)
    return ys, yv.reshape(n, -1)


def _np_tv_norm(xs, xv3):
    n = xs.shape[0]
    xs = xs / np.sqrt((xs ** 2).sum(axis=1, keepdims=True) + EPS_TV)
    xv = xv3.reshape(n, -1, 3)
    norm1 = np.sqrt((xv ** 2).sum(axis=1) + EPS_TV)   # [n, 3]
    nm = norm1.mean(axis=1)
    xv = xv / (nm[:, None, None] + EPS_TV)
    return xs, xv.reshape(n, -1)


def reference_numpy(x, B1, B2):
    xs = np.tanh(x[:, :MI])
    z1 = np.concatenate([xs, x[:, MI:]], axis=1)
    ys, yv = _np_layer(z1, B1, MI, MI, MH)
    ys, yv = _np_si_norm(ys, yv)
    ys, yv = _np_tv_norm(ys, yv)
    z2 = np.concatenate([ys, yv], axis=1)
    zs, zv = _np_layer(z2, B2, MH, MH, MO)
    zs, zv = _np_si_norm(zs, zv)
    zs = 1.0 / (1.0 + np.exp(-zs))
    return np.concatenate([zs, zv], axis=1).astype(np.float32)


# ---------------------------------------------------------------------------
# device kernel
# ---------------------------------------------------------------------------

_PROGRAM_CACHE = {}


def _emit_layer_set(nc, pools, src_ap_full, t0, S, nf_stride, vneg_full, bundle_rep,
                    ztall, ms, mv, mw, ys_macro, yv_macro):
    """One set of S (<=3) 128-row chunks of one tensor-product layer.

    src_ap_full: [128, T*nf_stride] SBUF macro tile holding z per chunk
    vneg_full:   [128, T*3mv] negated v features macro tile
    ys_macro:    [128, T*mw] tile; yv_macro: [128, T*3mw]
    """
    import concourse.mybir as mybir
    f32 = mybir.dt.float32
    nf = ms + 3 * mv
    q_s = ms + 3 * mv
    oC = mw * q_s
    oD = oC + mv * mw
    scols = mw * q_s

    pm = pools["psum_m"].tile([P, 3 * 1024], f32, tag="pm")
    npack = P // nf
    for c in range(S):
        t = t0 + c
        r0 = nf * (t % npack)
        lhsT = ztall[r0:r0 + nf, (t // npack) * P:(t // npack) * P + P]
        rhs0 = bundle_rep[r0:r0 + nf, 0:512]
        rhs1 = bundle_rep[r0:r0 + nf, 512:1024]
        tp = (r0, 0)
        nc.tensor.matmul(pm[:, c * 1024:c * 1024 + 512], lhsT, rhs0,
                         start=True, stop=True, tile_position=tp)
        nc.tensor.matmul(pm[:, c * 1024 + 512:c * 1024 + 1024], lhsT, rhs1,
                         start=True, stop=True, tile_position=tp)

    prod = pools["prod"].tile([P, 3 * 1664], f32, tag="prod")
    pm3 = pm[:, 0:S * 1024].rearrange("p (c n) -> p c n", c=S)
    pr3 = prod[:, 0:S * 1664].rearrange("p (c r) -> p c r", c=S)
    zsp = src_ap_full[:, t0 * nf_stride:(t0 + S - 1) * nf_stride + nf] \
        if nf_stride != nf else src_ap_full[:, t0 * nf:(t0 + S) * nf]
    # build [p, c, f] view of z across the set (chunk stride nf_stride)
    zc = src_ap_full[:, t0 * nf_stride:(t0 + S) * nf_stride].rearrange(
        "p (c f) -> p c f", c=S)[:, :, 0:nf]
    vc = zc[:, :, ms:nf]
    vnegc = vneg_full[:, t0 * 3 * mv:(t0 + S) * 3 * mv].rearrange(
        "p (c f) -> p c f", c=S)

    spart = pr3[:, :, 0:scols].rearrange("p c (w q) -> p c w q", w=mw)
    vpart = pr3[:, :, scols:1664].rearrange("p c (w k q) -> p c w k q", w=mw, k=3)

    # merged A+B products: z_q * M[w, q]
    nc.vector.tensor_mul(
        spart,
        pm3[:, :, 0:scols].rearrange("p c (w q) -> p c w q", w=mw),
        zc.unsqueeze(2).broadcast_to([P, S, mw, q_s]),
    )
    # C products: v[m,k] * C[m,w]  (one op per k: ISA caps free dims at 3)
    Cv = pm3[:, :, oC:oD].rearrange("p c (m w) -> p c w m", m=mv)
    vck = vc.rearrange("p c (m k) -> p c k m", m=mv)
    for k in range(3):
        nc.vector.tensor_mul(
            vpart[:, :, :, k, 0:mv],
            Cv,
            vck[:, :, k].unsqueeze(2).broadcast_to([P, S, mw, mv]),
        )
    # D products
    Dv = pm3[:, :, oD:1024].rearrange("p c (u j w) -> p c j w u", u=mv, j=3)
    vpos = vc.rearrange("p c (u i) -> p c i u", u=mv)
    vneg = vnegc.rearrange("p c (u i) -> p c i u", u=mv)
    for k in range(3):
        j1, i1 = (k + 2) % 3, (k + 1) % 3
        j2, i2 = (k + 1) % 3, (k + 2) % 3
        nc.vector.tensor_mul(
            vpart[:, :, :, k, mv:2 * mv],
            Dv[:, :, j1],
            vpos[:, :, i1].unsqueeze(2).broadcast_to([P, S, mw, mv]),
        )
        nc.vector.tensor_mul(
            vpart[:, :, :, k, 2 * mv:3 * mv],
            Dv[:, :, j2],
            vneg[:, :, i2].unsqueeze(2).broadcast_to([P, S, mw, mv]),
        )

    nc.vector.tensor_reduce(
        ys_macro[:, t0 * mw:(t0 + S) * mw].rearrange("p (c w) -> p c w", c=S),
        spart, axis=mybir.AxisListType.X, op=mybir.AluOpType.add)
    nc.vector.tensor_reduce(
        yv_macro[:, t0 * 3 * mw:(t0 + S) * 3 * mw].rearrange("p (c wk) -> p c wk", c=S),
        vpart.rearrange("p c w k q -> p c (w k) q"),
        axis=mybir.AxisListType.X, op=mybir.AluOpType.add)


def _emit_program(nc, tc, x_d, b1_d, b2_d, out_d, rows, repeat=1):
    import concourse.mybir as mybir
    from concourse.masks import make_identity
    AF = mybir.ActivationFunctionType
    ALU = mybir.AluOpType
    AX = mybir.AxisListType
    f32 = mybir.dt.float32
    from contextlib import ExitStack

    nmacro = rows // MACRO
    ctx = ExitStack()
    with ctx:
        cpool = ctx.enter_context(tc.tile_pool(name="const", bufs=1))
        ident = cpool.tile([P, P], f32)
        make_identity(nc, ident[:])

        def _bias_tile(val, idx=[0]):
            bt = cpool.tile([P, 1], f32, tag=f"bias{idx[0]}")
            idx[0] += 1
            nc.gpsimd.memset(bt[:], float(val))
            return bt[:]

        b_tiny = _bias_tile(TINY)
        b_si = _bias_tile(EPS_SI)
        b_tv = _bias_tile(EPS_TV)
        b_v1 = _bias_tile(MH * EPS_SI / (MH - 1) + TINY)
        b_v2 = _bias_tile(MO * EPS_SI / (MO - 1) + TINY)
        b1_sb = cpool.tile([P, 1024], f32)
        for r in range(4):
            nc.sync.dma_start(out=b1_sb[32 * r:32 * (r + 1), :], in_=b1_d[:])
        b2_sb = cpool.tile([P, 1024], f32)
        for r in range(2):
            nc.sync.dma_start(out=b2_sb[64 * r:64 * (r + 1), :], in_=b2_d[:])

        io_pool = ctx.enter_context(tc.tile_pool(name="io", bufs=2))
        mid_pool = ctx.enter_context(tc.tile_pool(name="mid", bufs=2))
        nrm_pool = ctx.enter_context(tc.tile_pool(name="nrm", bufs=2))
        zt_pool = ctx.enter_context(tc.tile_pool(name="zt", bufs=2))
        prod_pool = ctx.enter_context(tc.tile_pool(name="prod", bufs=2))
        psum_t = ctx.enter_context(tc.tile_pool(name="psum_t", bufs=2, space="PSUM"))
        psum_m = ctx.enter_context(tc.tile_pool(name="psum_m", bufs=1, space="PSUM"))
        pools = {"psum_t": psum_t, "psum_m": psum_m, "zt": zt_pool, "prod": prod_pool}

        for m in list(range(nmacro)) * repeat:
            xm = io_pool.tile([P, T * 32], f32, tag="xm")
            xview = x_d[m * MACRO:(m + 1) * MACRO, :].rearrange("(t p) f -> p t f", p=P)
            nc.sync.dma_start(out=xm[:].rearrange("p (t f) -> p t f", f=32), in_=xview)
            # tanh on scalar features (in place)
            xs_view = xm[:].rearrange("p (t f) -> p t f", f=32)[:, :, 0:MI]
            nc.scalar.activation(xs_view, xs_view, AF.Tanh)
            # negated vector features for the minus-branch of the cross products
            zneg1 = io_pool.tile([P, T * 24], f32, tag="zneg1")
            nc.vector.tensor_scalar_mul(
                zneg1[:].rearrange("p (t q) -> p t q", q=24),
                xm[:].rearrange("p (t f) -> p t f", f=32)[:, :, MI:32],
                -1.0,
            )

            ztall1 = zt_pool.tile([P, (T // 4) * P], f32, tag="ztall1")
            for g in range(T // 4):
                pt = psum_t.tile([P, P], f32, tag="pt")
                nc.tensor.transpose(pt[:], xm[:, g * P:(g + 1) * P], ident[:])
                nc.vector.tensor_copy(ztall1[:, g * P:(g + 1) * P], pt[:])
            ys8 = nrm_pool.tile([P, T * MH], f32, tag="ys8")
            yv8 = nrm_pool.tile([P, T * 3 * MH], f32, tag="yv8")
            t0 = 0
            while t0 < T:
                S = min(3, T - t0)
                _emit_layer_set(nc, pools, xm[:], t0, S, 32, zneg1[:],
                                b1_sb[:], ztall1[:], MI, MI, MH, ys8[:], yv8[:])
                t0 += S

            # ---- si_norm(1) + tv_norm, batched over T chunks ----
            mh = MH
            ys8v = ys8[:].rearrange("p (t f) -> p t f", f=mh)
            yv8v = yv8[:].rearrange("p (t f) -> p t f", f=3 * mh)
            sq_s = nrm_pool.tile([P, T * mh], f32, tag="sq_s")
            nc.scalar.activation(sq_s[:], ys8[:], AF.Square)
            sumsq_s = nrm_pool.tile([P, T], f32, tag="n1a")
            nc.vector.tensor_reduce(sumsq_s[:], sq_s[:].rearrange("p (t f) -> p t f", f=mh),
                                    axis=AX.X, op=ALU.add)
            sum_s = nrm_pool.tile([P, T], f32, tag="n1b")
            nc.vector.tensor_reduce(sum_s[:], ys8v, axis=AX.X, op=ALU.add)
            s2 = nrm_pool.tile([P, T], f32, tag="n1c")
            nc.scalar.activation(s2[:], sum_s[:], AF.Square, scale=float(mh) ** -0.5)
            varnum = nrm_pool.tile([P, T], f32, tag="n1d")
            nc.vector.tensor_sub(varnum[:], sumsq_s[:], s2[:])
            std_s = nrm_pool.tile([P, T], f32, tag="n1e")
            nc.scalar.activation(std_s[:], varnum[:], AF.Sqrt,
                                 scale=1.0 / (mh - 1), bias=b_tiny)
            stde_s = nrm_pool.tile([P, T], f32, tag="n1f")
            nc.vector.tensor_scalar_add(stde_s[:], std_s[:], EPS_SI)
            inv_s = nrm_pool.tile([P, T], f32, tag="n1g")
            nc.vector.reciprocal(inv_s[:], stde_s[:])

            sq_v = nrm_pool.tile([P, T * 3 * mh], f32, tag="sq_v")
            nc.scalar.activation(sq_v[:], yv8[:], AF.Square)
            n2u = nrm_pool.tile([P, T * mh], f32, tag="n2u")
            nc.vector.tensor_reduce(
                n2u[:], sq_v[:].rearrange("p (tu k) -> p tu k", k=3),
                axis=AX.X, op=ALU.add)
            norm1 = nrm_pool.tile([P, T * mh], f32, tag="norm1")
            nc.scalar.activation(norm1[:], n2u[:], AF.Sqrt, bias=b_si)
            rn = nrm_pool.tile([P, T], f32, tag="n1h")
            nc.vector.tensor_reduce(rn[:], n2u[:].rearrange("p (t u) -> p t u", u=mh),
                                    axis=AX.X, op=ALU.add)
            sum_n = nrm_pool.tile([P, T], f32, tag="n1i")
            nc.vector.tensor_reduce(sum_n[:], norm1[:].rearrange("p (t u) -> p t u", u=mh),
                                    axis=AX.X, op=ALU.add)
            s2n = nrm_pool.tile([P, T], f32, tag="n1j")
            nc.scalar.activation(s2n[:], sum_n[:], AF.Square, scale=float(mh) ** -0.5)
            varn = nrm_pool.tile([P, T], f32, tag="n1k")
            nc.vector.tensor_sub(varn[:], rn[:], s2n[:])
            std_v = nrm_pool.tile([P, T], f32, tag="n1l")
            nc.scalar.activation(std_v[:], varn[:], AF.Sqrt, scale=1.0 / (mh - 1),
                                 bias=b_v1)
            stde_v = nrm_pool.tile([P, T], f32, tag="n1m")
            nc.vector.tensor_scalar_add(stde_v[:], std_v[:], EPS_SI)
            inv_v = nrm_pool.tile([P, T], f32, tag="n1n")
            nc.vector.reciprocal(inv_v[:], stde_v[:])

            # tv_norm scalars
            invs2 = nrm_pool.tile([P, T], f32, tag="n1o")
            nc.scalar.activation(invs2[:], inv_s[:], AF.Square)
            q_s = nrm_pool.tile([P, T], f32, tag="n1p")
            nc.vector.tensor_mul(q_s[:], sumsq_s[:], invs2[:])
            rt_s = nrm_pool.tile([P, T], f32, tag="n1q")
            nc.scalar.activation(rt_s[:], q_s[:], AF.Sqrt, bias=b_tv)
            invt_s = nrm_pool.tile([P, T], f32, tag="n1r")
            nc.vector.reciprocal(invt_s[:], rt_s[:])
            scale_s = nrm_pool.tile([P, T], f32, tag="n1s")
            nc.vector.tensor_mul(scale_s[:], inv_s[:], invt_s[:])

            ni_raw = nrm_pool.tile([P, T * 3], f32, tag="n1t")
            nc.vector.tensor_reduce(
                ni_raw[:].rearrange("p (t i) -> p t i", i=3),
                sq_v[:].rearrange("p (t u i) -> p t i u", u=mh, i=3),
                axis=AX.X, op=ALU.add)
            iv2 = nrm_pool.tile([P, T], f32, tag="n1u")
            nc.scalar.activation(iv2[:], inv_v[:], AF.Square)
            ni_tv = nrm_pool.tile([P, T * 3], f32, tag="n1v")
            nc.vector.tensor_mul(
                ni_tv[:].rearrange("p (t i) -> p t i", i=3),
                ni_raw[:].rearrange("p (t i) -> p t i", i=3),
                iv2[:].unsqueeze(2).broadcast_to([P, T, 3]))
            n1tv = nrm_pool.tile([P, T * 3], f32, tag="n1w")
            nc.scalar.activation(n1tv[:], ni_tv[:], AF.Sqrt, bias=b_tv)
            nm3 = nrm_pool.tile([P, T], f32, tag="n1x")
            nc.vector.tensor_reduce(nm3[:], n1tv[:].rearrange("p (t i) -> p t i", i=3),
                                    axis=AX.X, op=ALU.add)
            nme = nrm_pool.tile([P, T], f32, tag="n1y")
            nc.vector.tensor_scalar(nme[:], nm3[:], 1.0 / 3.0, EPS_TV,
                                    op0=ALU.mult, op1=ALU.add)
            invtv = nrm_pool.tile([P, T], f32, tag="n1z")
            nc.vector.reciprocal(invtv[:], nme[:])
            scale_v = nrm_pool.tile([P, T], f32, tag="n1A")
            nc.vector.tensor_mul(scale_v[:], inv_v[:], invtv[:])

            # apply scales -> zmid [128, T*64]; also negated v for L2 D-products
            zmid = mid_pool.tile([P, T * 64], f32, tag="zmid")
            zmv = zmid[:].rearrange("p (t f) -> p t f", f=64)
            nc.vector.tensor_mul(
                zmv[:, :, 0:MH], ys8v,
                scale_s[:].unsqueeze(2).broadcast_to([P, T, MH]))
            nc.vector.tensor_mul(
                zmv[:, :, MH:64], yv8v,
                scale_v[:].unsqueeze(2).broadcast_to([P, T, 3 * MH]))
            zneg2 = mid_pool.tile([P, T * 48], f32, tag="zneg2")
            nscale_v = nrm_pool.tile([P, T], f32, tag="n1B")
            nc.vector.tensor_scalar_mul(nscale_v[:], scale_v[:], -1.0)
            nc.vector.tensor_mul(
                zneg2[:].rearrange("p (t f) -> p t f", f=48), yv8v,
                nscale_v[:].unsqueeze(2).broadcast_to([P, T, 48]))

            # ---- layer 2 ----
            ztall2 = zt_pool.tile([P, (T // 2) * P], f32, tag="ztall2")
            for g in range(T // 2):
                pt = psum_t.tile([P, P], f32, tag="pt")
                nc.tensor.transpose(pt[:], zmid[:, g * P:(g + 1) * P], ident[:])
                nc.vector.tensor_copy(ztall2[:, g * P:(g + 1) * P], pt[:])
            zs8 = nrm_pool.tile([P, T * MO], f32, tag="zs8")
            zv8 = nrm_pool.tile([P, T * 3 * MO], f32, tag="zv8")
            t0 = 0
            while t0 < T:
                S = min(3, T - t0)
                _emit_layer_set(nc, pools, zmid[:], t0, S, 64, zneg2[:],
                                b2_sb[:], ztall2[:], MH, MH, MO, zs8[:], zv8[:])
                t0 += S

            # ---- si_norm(2) ----
            mo = MO
            zs8v = zs8[:].rearrange("p (t f) -> p t f", f=mo)
            zv8v = zv8[:].rearrange("p (t f) -> p t f", f=3 * mo)
            sq_s2 = nrm_pool.tile([P, T * mo], f32, tag="sq_s2")
            nc.scalar.activation(sq_s2[:], zs8[:], AF.Square)
            sumsq2 = nrm_pool.tile([P, T], f32, tag="n2a")
            nc.vector.tensor_reduce(sumsq2[:], sq_s2[:].rearrange("p (t f) -> p t f", f=mo),
                                    axis=AX.X, op=ALU.add)
            sum2 = nrm_pool.tile([P, T], f32, tag="n2b")
            nc.vector.tensor_reduce(sum2[:], zs8v, axis=AX.X, op=ALU.add)
            s22 = nrm_pool.tile([P, T], f32, tag="n2c")
            nc.scalar.activation(s22[:], sum2[:], AF.Square, scale=float(mo) ** -0.5)
            varnum2 = nrm_pool.tile([P, T], f32, tag="n2d")
            nc.vector.tensor_sub(varnum2[:], sumsq2[:], s22[:])
            std_s2 = nrm_pool.tile([P, T], f32, tag="n2e")
            nc.scalar.activation(std_s2[:], varnum2[:], AF.Sqrt,
                                 scale=1.0 / (mo - 1), bias=b_tiny)
            stde_s2 = nrm_pool.tile([P, T], f32, tag="n2f")
            nc.vector.tensor_scalar_add(stde_s2[:], std_s2[:], EPS_SI)
            inv_s2 = nrm_pool.tile([P, T], f32, tag="n2g")
            nc.vector.reciprocal(inv_s2[:], stde_s2[:])

            sq_v2 = nrm_pool.tile([P, T * 3 * mo], f32, tag="sq_v2")
            nc.scalar.activation(sq_v2[:], zv8[:], AF.Square)
            n2u2 = nrm_pool.tile([P, T * mo], f32, tag="n2u2")
            nc.vector.tensor_reduce(n2u2[:], sq_v2[:].rearrange("p (tu k) -> p tu k", k=3),
                                    axis=AX.X, op=ALU.add)
            norm12 = nrm_pool.tile([P, T * mo], f32, tag="norm12")
            nc.scalar.activation(norm12[:], n2u2[:], AF.Sqrt, bias=b_si)
            rn2 = nrm_pool.tile([P, T], f32, tag="n2h")
            nc.vector.tensor_reduce(rn2[:], n2u2[:].rearrange("p (t u) -> p t u", u=mo),
                                    axis=AX.X, op=ALU.add)
            sum_n2 = nrm_pool.tile([P, T], f32, tag="n2i")
            nc.vector.tensor_reduce(sum_n2[:], norm12[:].rearrange("p (t u) -> p t u", u=mo),
                                    axis=AX.X, op=ALU.add)
            s2n2 = nrm_pool.tile([P, T], f32, tag="n2j")
            nc.scalar.activation(s2n2[:], sum_n2[:], AF.Square, scale=float(mo) ** -0.5)
            varn2 = nrm_pool.tile([P, T], f32, tag="n2k")
            nc.vector.tensor_sub(varn2[:], rn2[:], s2n2[:])
            std_v2 = nrm_pool.tile([P, T], f32, tag="n2l")
            nc.scalar.activation(std_v2[:], varn2[:], AF.Sqrt, scale=1.0 / (mo - 1),
                                 bias=b_v2)
            stde_v2 = nrm_pool.tile([P, T], f32, tag="n2m")
            nc.vector.tensor_scalar_add(stde_v2[:], std_v2[:], EPS_SI)
            inv_v2 = nrm_pool.tile([P, T], f32, tag="n2n")
            nc.vector.reciprocal(inv_v2[:], stde_v2[:])

            # scale, sigmoid, assemble output macro [128, T*32]
            outm = io_pool.tile([P, T * 32], f32, tag="outm")
            outv = outm[:].rearrange("p (t f) -> p t f", f=32)
            tmp_s = nrm_pool.tile([P, T * mo], f32, tag="tmp_s")
            nc.vector.tensor_mul(
                tmp_s[:].rearrange("p (t f) -> p t f", f=mo), zs8v,
                inv_s2[:].unsqueeze(2).broadcast_to([P, T, mo]))
            nc.scalar.activation(outv[:, :, 0:MO],
                                 tmp_s[:].rearrange("p (t f) -> p t f", f=mo),
                                 AF.Sigmoid)
            nc.vector.tensor_mul(
                outv[:, :, MO:32], zv8v,
                inv_v2[:].unsqueeze(2).broadcast_to([P, T, 3 * mo]))

            oview = out_d[m * MACRO:(m + 1) * MACRO, :].rearrange("(t p) f -> p t f", p=P)
            nc.sync.dma_start(out=oview, in_=outm[:].rearrange("p (t f) -> p t f", f=32))


def _build_program(rows, repeat=1):
    import concourse.bacc as bacc
    import concourse.tile as tile
    import concourse.mybir as mybir
    f32 = mybir.dt.float32

    nc = bacc.Bacc("TRN2", target_bir_lowering=False, debug=False,
                   enable_asserts=False, num_devices=NCORES)
    x_d = nc.dram_tensor("x", [rows, 32], f32, kind="ExternalInput").ap()
    b1_d = nc.dram_tensor("b1", [32, 1024], f32, kind="ExternalInput").ap()
    b2_d = nc.dram_tensor("b2", [64, 1024], f32, kind="ExternalInput").ap()
    out_d = nc.dram_tensor("out", [rows, 32], f32, kind="ExternalOutput").ap()

    with tile.TileContext(nc) as tc:
        _emit_program(nc, tc, x_d, b1_d, b2_d, out_d, rows, repeat)
    nc.compile()
    return nc


def _get_program(rows, repeat=1):
    key = (rows, repeat)
    if key not in _PROGRAM_CACHE:
        _PROGRAM_CACHE[key] = _build_program(rows, repeat)
    return _PROGRAM_CACHE[key]


_RUN_CACHE = {}


def _get_runner(rows, repeat):
    """Build (once) a cached jitted shard_map executable for the program."""
    key = (rows, repeat)
    if key in _RUN_CACHE:
        return _RUN_CACHE[key]
    import jax
    import numpy as _np
    from jax.sharding import Mesh, PartitionSpec
    try:
        from jax.experimental.shard_map import shard_map
    except Exception:
        from jax.shard_map import shard_map  # newer jax
    from concourse import bass2jax
    import concourse.mybir as mybir

    nc = _get_program(rows, repeat)
    bass2jax.install_neuronx_cc_hook()
    partition_name = nc.partition_id_tensor.name if nc.partition_id_tensor else None
    in_names, out_names, out_avals, zero_outs = [], [], [], []
    for alloc in nc.m.functions[0].allocations:
        if not isinstance(alloc, mybir.MemoryLocationSet):
            continue
        name = alloc.memorylocations[0].name
        if alloc.kind == "ExternalInput":
            if name != partition_name:
                in_names.append(name)
        elif alloc.kind == "ExternalOutput":
            shape = tuple(alloc.tensor_shape)
            dtype = mybir.dt.np(alloc.dtype)
            out_names.append(name)
            out_avals.append(jax.core.ShapedArray(shape, dtype))
            zero_outs.append(_np.zeros(shape, dtype))
    n_params = len(in_names)
    n_outs = len(out_avals)
    all_in_names = list(in_names) + list(out_names)
    if partition_name is not None:
        all_in_names.append(partition_name)
    donate = tuple(range(n_params, n_params + n_outs))

    def _body(*args):
        operands = list(args)
        if partition_name is not None:
            operands.append(bass2jax.partition_id_tensor())
        outs = bass2jax._bass_exec_p.bind(
            *operands,
            out_avals=tuple(out_avals),
            in_names=tuple(all_in_names),
            out_names=tuple(out_names),
            lowering_input_output_aliases=(),
            sim_require_finite=True,
            sim_require_nnan=True,
            nc=nc,
        )
        return tuple(outs)

    devices = jax.devices()[:NCORES]
    mesh = Mesh(_np.asarray(devices), ("core",))
    in_specs = (PartitionSpec("core"),) * (n_params + n_outs)
    out_specs = (PartitionSpec("core"),) * n_outs
    sharded = jax.jit(
        shard_map(_body, mesh=mesh, in_specs=in_specs, out_specs=out_specs,
                  check_rep=False),
        donate_argnums=donate, keep_unused=True,
    )
    runner = (sharded, in_names, out_names, out_avals, zero_outs)
    _RUN_CACHE[key] = runner
    return runner


_ZERO_CACHE = {}


def _run_cached(rows, repeat, full_inputs):
    """full_inputs: dict name -> already-concatenated (NCORES*rows0, ...) array."""
    import numpy as _np
    sharded, in_names, out_names, out_avals, zero_outs = _get_runner(rows, repeat)
    concat_in = [full_inputs[nm] for nm in in_names]
    key = (rows, repeat)
    if key not in _ZERO_CACHE:
        _ZERO_CACHE[key] = [
            _np.zeros((NCORES * z.shape[0], *z.shape[1:]), z.dtype)
            for z in zero_outs]
    out_arrs = sharded(*concat_in, *_ZERO_CACHE[key])
    i = out_names.index("out")
    return _np.asarray(out_arrs[i]).reshape(NCORES, *out_avals[i].shape)


def kernel(x, w1_ss, w1_vv0, w1_sv, w1_vs, w1_vv1,
           w2_ss, w2_vv0, w2_sv, w2_vs, w2_vv1, _trace=False, _repeat=1):
    from concourse import bass_utils

    x = np.asarray(x, dtype=np.float32)
    B1 = _build_bundle(MI, MI, MH, np.asarray(w1_ss), np.asarray(w1_vv0),
                       np.asarray(w1_sv), np.asarray(w1_vs), np.asarray(w1_vv1))
    B2 = _build_bundle(MH, MH, MO, np.asarray(w2_ss), np.asarray(w2_vv0),
                       np.asarray(w2_sv), np.asarray(w2_vs), np.asarray(w2_vv1))

    rows = x.shape[0] // NCORES
    if _trace:
        shards = x.reshape(NCORES, rows, 32)
        in_maps = [{"x": np.ascontiguousarray(shards[i]), "b1": B1, "b2": B2}
                   for i in range(NCORES)]
        nc = _get_program(rows, _repeat)
        res = bass_utils.run_bass_kernel_spmd(nc, in_maps,
                                              core_ids=list(range(NCORES)),
                                              trace=True)
        out = np.concatenate([res.results[i]["out"] for i in range(NCORES)], axis=0)
        return out, res
    full_inputs = {"x": np.ascontiguousarray(x),
                   "b1": np.tile(B1, (NCORES, 1)),
                   "b2": np.tile(B2, (NCORES, 1))}
    try:
        per_core = _run_cached(rows, _repeat, full_inputs)
        return np.ascontiguousarray(per_core.reshape(rows * NCORES, 32))
    except Exception:
        shards = x.reshape(NCORES, rows, 32)
        in_maps = [{"x": np.ascontiguousarray(shards[i]), "b1": B1, "b2": B2}
                   for i in range(NCORES)]
        nc = _get_program(rows, _repeat)
        res = bass_utils.run_bass_kernel_spmd(nc, in_maps,
                                              core_ids=list(range(NCORES)))
        return np.concatenate([res.results[i]["out"] for i in range(NCORES)], axis=0)
